# revision 1
# baseline (speedup 1.0000x reference)
"""AlphaStock Trainium2 kernel (8 NeuronCores, SPMD).

Model: per-asset LSTM(T=64, H=128) + temporal attention pooling (HA), then
cross-asset attention (CAAN) over M=512 assets per batch element.

Sharding: the B*M=1024 sequences are split 128-per-core for the LSTM/HA
stage (launch A). The tiny CAAN stage runs as a second launch (B) sharded
by query rows (cores 0-3 -> batch 0, cores 4-7 -> batch 1), with the
gathered per-asset representations re-broadcast by the host between the
two launches.

Layouts (launch A, per core, n = 128 local sequences):
  - gates PSUM tile [128 units, 512] per step, gate order (i, f, o, g)
  - h stored transposed: rep[h, (n, t)]  (free index = n*64 + t)
  - x pre-transposed on host: xT[f(+1 ones row), t*128+n]; the ones row
    carries b_ih+b_hh through the x-matmul so no separate bias add exists.
"""

import numpy as np

B, M, T, F, H = 2, 512, 64, 16, 128
N_CORES = 8
NPC = (B * M) // N_CORES  # sequences per core = 128
G4 = 4 * H  # 512

_CACHE = {}


def _gate_perm():
    # torch gate order (i, f, g, o) -> kernel order (i, f, o, g)
    idx = np.arange(4 * H).reshape(4, H)
    return np.concatenate([idx[0], idx[1], idx[3], idx[2]])


def _build_launch_a():
    import concourse.bacc as bacc
    import concourse.tile as tile
    import concourse.bass as bass
    from concourse import mybir
    from contextlib import ExitStack

    f32 = mybir.dt.float32
    f16 = mybir.dt.float16
    AF = mybir.ActivationFunctionType

    nc = bacc.Bacc("TRN2", target_bir_lowering=False, debug=False,
                   num_devices=N_CORES)

    xT = nc.dram_tensor("xT", [F + 1, T * NPC], f16, kind="ExternalInput").ap()
    wih = nc.dram_tensor("wih", [F + 1, G4], f16, kind="ExternalInput").ap()
    whh = nc.dram_tensor("whh", [H, G4], f16, kind="ExternalInput").ap()
    w1T = nc.dram_tensor("w1T", [H, H], f16, kind="ExternalInput").ap()
    w2T = nc.dram_tensor("w2T", [H, H], f16, kind="ExternalInput").ap()
    b12 = nc.dram_tensor("b12", [H, 1], f32, kind="ExternalInput").ap()
    waT = nc.dram_tensor("waT", [H, H], f16, kind="ExternalInput").ap()
    stock = nc.dram_tensor("stock", [H, NPC], f32, kind="ExternalOutput").ap()

    with tile.TileContext(nc) as tc, ExitStack() as ctx:
        big = ctx.enter_context(tc.tile_pool(name="big", bufs=1))
        state = ctx.enter_context(tc.tile_pool(name="state", bufs=1))
        work = ctx.enter_context(tc.tile_pool(name="work", bufs=3))
        psum = ctx.enter_context(tc.tile_pool(name="psum", bufs=4, space="PSUM"))
        psumw = ctx.enter_context(tc.tile_pool(name="psumw", bufs=1, space="PSUM"))
        dram = ctx.enter_context(tc.tile_pool(name="dram", bufs=1, space="DRAM"))

        # ---- resident tensors
        xsb = big.tile([F + 1, T * NPC], f16, tag="xsb")
        rep = big.tile([H, NPC * T], f16, tag="rep")     # rep[h, n*64+t]

        wih_sb = state.tile([F + 1, G4], f16, tag="wih")
        whh_sb = state.tile([H, G4], f16, tag="whh")
        w1T_sb = state.tile([H, H], f16, tag="w1T")
        w2T_sb = state.tile([H, H], f16, tag="w2T")
        b12_sb = state.tile([H, 1], f32, tag="b12")
        waT_sb = state.tile([H, H], f16, tag="waT")
        tgc = state.tile([H, 2 * H], f16, tag="tgc")     # [tanh(g) | c]

        nc.sync.dma_start(out=wih_sb, in_=wih)
        nc.sync.dma_start(out=whh_sb, in_=whh)
        # x arrives in 8 time-chunks so the recurrence starts after ~1/8th
        # of the transfer (and the chunks ride parallel DMA queues)
        XCH = T * NPC // 8
        for j in range(8):
            nc.sync.dma_start(out=xsb[:, j * XCH:(j + 1) * XCH],
                              in_=xT[:, j * XCH:(j + 1) * XCH])
        nc.sync.dma_start(out=w1T_sb, in_=w1T)
        nc.sync.dma_start(out=w2T_sb, in_=w2T)
        nc.sync.dma_start(out=b12_sb, in_=b12)
        nc.sync.dma_start(out=waT_sb, in_=waT)
        nc.vector.memset(tgc[:, H:2 * H], 0.0)  # c0 = 0

        rep3 = rep.rearrange("p (n t) -> p n t", t=T)

        # ---- LSTM over T steps.
        # Engine streams execute in program order: x-matmuls are emitted
        # XAHEAD steps early so the PE runs them while it waits for h_{t-1},
        # and only the 4 W_hh matmuls sit on the recurrence critical path.
        XAHEAD = 3
        ps_tiles = {}

        def emit_x(t):
            ps = psum.tile([H, G4], f32, tag="gates")
            ps_tiles[t] = ps
            for g in range(4):
                gs = slice(g * H, (g + 1) * H)
                # g==0 start=True zeroes the whole 2KB bank (zero region);
                # gates 1-3 land on pending-zero bytes and overwrite.
                nc.tensor.matmul(ps[:, gs], lhsT=wih_sb[:, gs],
                                 rhs=xsb[:, t * NPC:(t + 1) * NPC],
                                 start=(g == 0), stop=True,
                                 skip_group_check=(g != 0))

        h_prev = None
        for t in range(XAHEAD):
            emit_x(t)
        for t in range(T):
            ps = ps_tiles.pop(t)
            if t > 0:
                for g in range(4):
                    gs = slice(g * H, (g + 1) * H)
                    nc.tensor.matmul(ps[:, gs], lhsT=whh_sb[:, gs],
                                     rhs=h_prev, start=False, stop=True,
                                     skip_group_check=True)
            if t + XAHEAD < T:
                emit_x(t + XAHEAD)
            sig = work.tile([H, 3 * H], f16, tag="sig")
            nc.scalar.activation(sig[:, 0:2 * H], ps[:, 0:2 * H], AF.Sigmoid)
            nc.scalar.activation(tgc[:, 0:H], ps[:, 3 * H:4 * H], AF.Tanh)
            prodt = work.tile([H, 2 * H], f16, tag="prodt")
            # [sig_i*tanh_g | sig_f*c]
            nc.vector.tensor_mul(prodt, sig[:, 0:2 * H], tgc)
            # sig_o computed while the DVE combines the cell state
            nc.scalar.activation(sig[:, 2 * H:3 * H], ps[:, 2 * H:3 * H],
                                 AF.Sigmoid)
            nc.vector.tensor_add(tgc[:, H:2 * H], prodt[:, 0:H], prodt[:, H:2 * H])
            tcc = work.tile([H, H], f16, tag="tcc")
            nc.scalar.activation(tcc, tgc[:, H:2 * H], AF.Tanh)
            h_cur = work.tile([H, NPC], f16, tag="hcur")
            nc.vector.tensor_mul(h_cur, sig[:, 2 * H:3 * H], tcc)
            # archive h into rep off the critical path (strided write)
            nc.gpsimd.tensor_copy(rep3[:, :, t], h_cur)
            h_prev = h_cur

        # ---- HA attention pooling, pipelined in 4 groups of 4 chunks.
        # The softmax over t is chunk-local (each chunk holds whole (n, t)
        # rows), so alpha/softmax/weighted-sum runs per group instead of as
        # a serial tail after all 16 chunks.
        CH = 512             # free elems per chunk
        NCH = NPC * T // CH  # 16 chunks
        NPCH = CH // T       # 8 sequences per chunk
        GRP = 4              # chunks per alpha group
        GN = GRP * NPCH      # 32 sequences per group
        GW = GRP * CH        # 2048 free elems per group
        hl = h_prev  # h_63, contiguous fp16 [H, NPC]
        stock_sb = state.tile([H, NPC], f32, tag="stock_sb")
        ssum = state.tile([H, NPC], f32, tag="ssum")
        rr = state.tile([H, NPC], f32, tag="rr")
        stku = state.tile([H, NPC], f32, tag="stku")
        for grp in range(NCH // GRP):
            for lc in range(GRP):
                ch = grp * GRP + lc
                cs = slice(ch * CH, (ch + 1) * CH)
                aps = psum.tile([H, CH], f32, tag="gates")
                nc.tensor.matmul(aps.rearrange("p (n t) -> p n t", t=T),
                                 lhsT=w1T_sb,
                                 rhs=rep[:, cs].rearrange("p (n t) -> p n t", t=T),
                                 start=True, stop=False)
                # a2 contribution: h_last broadcast over t via step-0 rhs AP
                hsl = hl[:, ch * NPCH:(ch + 1) * NPCH]
                hsl_b = bass.AP(tensor=hsl.tensor, offset=hsl.offset,
                                ap=[*hsl.ap, [0, T]])
                nc.tensor.matmul(aps.rearrange("p (n t) -> p n t", t=T),
                                 lhsT=w2T_sb, rhs=hsl_b,
                                 start=False, stop=True)
                z = work.tile([H, CH], f16, tag="z")
                nc.scalar.activation(z, aps, AF.Tanh, bias=b12_sb)
                if lc == 0:
                    wps = psumw.tile([H, GW], f32, tag="wps")
                nc.tensor.matmul(wps[:, lc * CH:(lc + 1) * CH], lhsT=waT_sb,
                                 rhs=z, start=True, stop=True)
            gsl = slice(grp * GW, (grp + 1) * GW)
            nsl = slice(grp * GN, (grp + 1) * GN)
            # wps rows are replicated across all 128 partitions, so the whole
            # softmax + weighted sum runs full-lane with no cross-partition
            # moves: exp, per-n sums, unnormalized stock, then one scale.
            eU = work.tile([H, GW], f16, tag="eU")
            nc.scalar.activation(eU, wps, AF.Exp)
            nc.vector.tensor_reduce(
                ssum[:, nsl], eU.rearrange("p (n t) -> p n t", t=T),
                mybir.AxisListType.X, mybir.AluOpType.add)
            nc.vector.reciprocal(rr[:, nsl], ssum[:, nsl])
            nc.vector.tensor_mul(eU, rep[:, gsl], eU)
            nc.vector.tensor_reduce(
                stku[:, nsl], eU.rearrange("p (n t) -> p n t", t=T),
                mybir.AxisListType.X, mybir.AluOpType.add)
            nc.vector.tensor_mul(stock_sb[:, nsl], stku[:, nsl], rr[:, nsl])
        nc.sync.dma_start(out=stock, in_=stock_sb)

    nc.compile()
    return nc


def _build_launch_b():
    import concourse.bacc as bacc
    import concourse.tile as tile
    from concourse import mybir
    from contextlib import ExitStack

    f32 = mybir.dt.float32
    f16 = mybir.dt.float16
    AF = mybir.ActivationFunctionType

    nc = bacc.Bacc("TRN2", target_bir_lowering=False, debug=False,
                   num_devices=N_CORES)

    xrT = nc.dram_tensor("xrT", [H, M], f16, kind="ExternalInput").ap()
    xqT = nc.dram_tensor("xqT", [H, NPC], f16, kind="ExternalInput").ap()
    # packed: [wqT | wkT | wvT | eye | wwT-col]
    wpk = nc.dram_tensor("wpk", [H, 4 * H + 1], f16, kind="ExternalInput").ap()
    # packed: [bq | bk | cst-broadcast-row]
    bpk = nc.dram_tensor("bpk", [H, 3], f32, kind="ExternalInput").ap()
    scores = nc.dram_tensor("scores", [1, NPC], f32, kind="ExternalOutput").ap()

    with tile.TileContext(nc) as tc, ExitStack() as ctx:
        pool = ctx.enter_context(tc.tile_pool(name="sb", bufs=1))
        psum = ctx.enter_context(tc.tile_pool(name="ps", bufs=1, space="PSUM"))

        xrT_sb = pool.tile([H, M], f16, tag="xrT")
        xqT_sb = pool.tile([H, NPC], f16, tag="xqT")
        wpk_sb = pool.tile([H, 4 * H + 1], f16, tag="wpk")
        bpk_sb = pool.tile([H, 3], f32, tag="bpk")
        nc.sync.dma_start(out=xrT_sb, in_=xrT)
        nc.sync.dma_start(out=xqT_sb, in_=xqT)
        nc.sync.dma_start(out=wpk_sb, in_=wpk)
        nc.sync.dma_start(out=bpk_sb, in_=bpk)
        wqT_sb = wpk_sb[:, 0:H]
        wkT_sb = wpk_sb[:, H:2 * H]
        wvT_sb = wpk_sb[:, 2 * H:3 * H]
        eye_sb = wpk_sb[:, 3 * H:4 * H]
        wwT_sb = wpk_sb[:, 4 * H:4 * H + 1]
        bq_sb = bpk_sb[:, 0:1]
        bk_sb = bpk_sb[:, 1:2]
        cst_sb = bpk_sb[:, 2:3]

        # q/k projections (transposed layout [h', *])
        qps = psum.tile([H, NPC], f32, tag="ps")
        nc.tensor.matmul(qps, lhsT=wqT_sb, rhs=xqT_sb, start=True, stop=True)
        qsb = pool.tile([H, NPC], f16, tag="qsb")
        nc.scalar.activation(qsb, qps, AF.Identity, bias=bq_sb)

        kps = psum.tile([H, M], f32, tag="kps")
        nc.tensor.matmul(kps, lhsT=wkT_sb, rhs=xrT_sb, start=True, stop=True)
        ksb = pool.tile([H, M], f16, tag="ksb")
        nc.scalar.activation(ksb, kps, AF.Identity, bias=bk_sb)

        # v in [k, h'] layout (no bias: beta rows sum to 1, folded into cst)
        vsb = pool.tile([H, 4, H], f16, tag="vsb")
        for j in range(4):
            vps = psum.tile([H, H], f32, tag="ps")
            nc.tensor.matmul(vps, lhsT=xrT_sb[:, j * H:(j + 1) * H],
                             rhs=wvT_sb, start=True, stop=True)
            nc.scalar.activation(vsb[:, j, :], vps, AF.Copy)

        # S = q^T k / sqrt(H); e = exp
        sps = psum.tile([NPC, M], f32, tag="sps")
        nc.tensor.matmul(sps, lhsT=qsb, rhs=ksb, start=True, stop=True)
        esb = pool.tile([NPC, M], f16, tag="esb")
        nc.scalar.activation(esb, sps, AF.Exp, scale=float(1.0 / np.sqrt(H)))
        ssum = pool.tile([NPC, 1], f32, tag="ssum")
        nc.vector.tensor_reduce(ssum, esb, mybir.AxisListType.X,
                                mybir.AluOpType.add)
        rr = pool.tile([NPC, 1], f32, tag="rr")
        nc.vector.reciprocal(rr, ssum)
        nc.vector.tensor_scalar_mul(esb, esb, rr)

        # transpose e chunks -> eT [k, q], then attnT = sum_j v_j @ eT_j
        eT = pool.tile([H, 4, NPC], f16, tag="eT")
        for j in range(4):
            tps = psum.tile([H, NPC], f16, tag="tp")
            nc.tensor.transpose(tps, esb[:, j * H:(j + 1) * H], eye_sb)
            nc.vector.tensor_copy(eT[:, j, :], tps)
        aps = psum.tile([H, NPC], f32, tag="aps")
        for j in range(4):
            nc.tensor.matmul(aps, lhsT=vsb[:, j, :], rhs=eT[:, j, :],
                             start=(j == 0), stop=(j == 3))
        attn = pool.tile([H, NPC], f16, tag="attn")
        nc.scalar.activation(attn, aps, AF.Copy)

        scps = psum.tile([1, NPC], f32, tag="scps")
        nc.tensor.matmul(scps, lhsT=wwT_sb, rhs=attn, start=True, stop=True)
        ssb = pool.tile([1, NPC], f32, tag="ssb")
        nc.scalar.activation(ssb, scps, AF.Identity, bias=cst_sb[0:1, :])
        nc.sync.dma_start(out=scores, in_=ssb)

    nc.compile()
    return nc


def _prep_inputs_a(inputs):
    perm = _gate_perm()
    W_ih = np.asarray(inputs["W_ih"])[perm]          # [512, 16]
    W_hh = np.asarray(inputs["W_hh"])[perm]          # [512, 128]
    bias = (np.asarray(inputs["b_ih"]) + np.asarray(inputs["b_hh"]))[perm]
    wih = np.concatenate([W_ih.T, bias[None, :]], axis=0)  # [17, 512]
    whh = np.ascontiguousarray(W_hh.T)               # [128, 512]
    w1T = np.ascontiguousarray(np.asarray(inputs["w1"]).T)
    w2T = np.ascontiguousarray(np.asarray(inputs["w2"]).T)
    b12 = (np.asarray(inputs["b1"]) + np.asarray(inputs["b2"]))[:, None]
    waT = np.repeat(np.asarray(inputs["wa"]).T, H, axis=1)  # [128, 128] replicated

    x = np.asarray(inputs["x"]).reshape(B * M, T, F)
    shared = dict(wih=np.ascontiguousarray(wih).astype(np.float16),
                  whh=whh.astype(np.float16),
                  w1T=w1T.astype(np.float16), w2T=w2T.astype(np.float16),
                  b12=np.ascontiguousarray(b12, np.float32),
                  waT=waT.astype(np.float16))
    in_maps = []
    for c in range(N_CORES):
        xc = x[c * NPC:(c + 1) * NPC]                # [128, 64, 16]
        xTc = np.empty((F + 1, T * NPC), np.float16)
        xTc[:F] = xc.transpose(2, 1, 0).reshape(F, T * NPC)  # [f, t*128+n]
        xTc[F] = 1.0
        in_maps.append(dict(xT=np.ascontiguousarray(xTc), **shared))
    return in_maps


def _prep_inputs_b(inputs, xr):
    # xr: [B, M, H] gathered stock_rep
    wqT = np.ascontiguousarray(np.asarray(inputs["wq"]).T).astype(np.float16)
    wkT = np.ascontiguousarray(np.asarray(inputs["wk"]).T).astype(np.float16)
    wvT = np.ascontiguousarray(np.asarray(inputs["wv"]).T).astype(np.float16)
    bq = np.ascontiguousarray(np.asarray(inputs["bq"])[:, None], np.float32)
    bk = np.ascontiguousarray(np.asarray(inputs["bk"])[:, None], np.float32)
    ww = np.asarray(inputs["ww"])                     # [1, H]
    bv = np.asarray(inputs["bv"])                     # [H]
    bw = np.asarray(inputs["bw"])                     # [1]
    wwT = np.ascontiguousarray(ww.T).astype(np.float16)
    cst = float(ww[0] @ bv + bw[0])
    eye = np.eye(H, dtype=np.float16)
    wpk = np.concatenate([wqT, wkT, wvT, eye, wwT], axis=1)
    bpk = np.concatenate([bq, bk, np.full((H, 1), cst, np.float32)], axis=1)
    wpk = np.ascontiguousarray(wpk)
    bpk = np.ascontiguousarray(bpk)

    in_maps = []
    for c in range(N_CORES):
        b, qc = c // 4, c % 4
        xrT = np.ascontiguousarray(xr[b].T).astype(np.float16)   # [H, M]
        xqT = np.ascontiguousarray(xrT[:, qc * NPC:(qc + 1) * NPC])
        in_maps.append(dict(xrT=xrT, xqT=xqT, wpk=wpk, bpk=bpk))
    return in_maps


def _get_programs():
    if "a" not in _CACHE:
        _CACHE["a"] = _build_launch_a()
    if "b" not in _CACHE:
        _CACHE["b"] = _build_launch_b()
    return _CACHE["a"], _CACHE["b"]


def _gather_xr(results_a):
    xr = np.empty((B, M, H), np.float32)
    for c in range(N_CORES):
        st = results_a[c]["stock"]                   # [H, NPC]
        n0 = c * NPC
        b, m0 = divmod(n0, M)
        xr[b, m0:m0 + NPC] = st.T
    return xr


def _assemble_scores(results_b):
    out = np.empty((B, M), np.float32)
    for c in range(N_CORES):
        b, qc = c // 4, c % 4
        out[b, qc * NPC:(qc + 1) * NPC] = results_b[c]["scores"][0]
    return out


def kernel(**inputs):
    from concourse.bass_utils import run_bass_kernel_spmd

    nca, ncb = _get_programs()
    in_maps_a = _prep_inputs_a(inputs)
    res_a = run_bass_kernel_spmd(nca, in_maps_a, core_ids=list(range(N_CORES)))
    xr = _gather_xr(res_a.results)
    in_maps_b = _prep_inputs_b(inputs, xr)
    res_b = run_bass_kernel_spmd(ncb, in_maps_b, core_ids=list(range(N_CORES)))
    return _assemble_scores(res_b.results)

